# revision 66
# baseline (speedup 1.0000x reference)
"""
Trainium2 Bass kernel for nn_CausalSelfAttention_5214090298017.

Reference computes (B=2, T=2048, C=768, H=12, HD=64):
    q,k,v = split_heads(x @ W{q,k,v}.T + b)          # [B,H,T,HD]
    att   = softmax(mask(q @ k.T / sqrt(HD)))        # key-padding mask from attn_mask1
    y     = (att @ v).merge_heads() @ Wp.T + bp      # [B,T,C]

Sharding: 8 cores = 2 (batch) x 4 (head-groups of 3 heads).  Each core
computes a partial output  sum_{h in group} (att_h @ v_h) @ Wp_rows_h
([T, C]); the host sums the 4 group partials per batch (row-parallel Wp)
and concatenates over batch.

Key layout trick: the host PERMUTES each batch's tokens so the unmasked
keys come first.  Q/K/V all project from the SAME [C, T] x^T tensor
(K/V read only the first tk columns), the key-padding mask reduces to
"key index >= n_valid", and the host un-permutes the output rows.

Device-side details (per core):
  - Q^T/K^T stored [head_dim, T]; heads 0/1 packed on partitions 0-63 /
    64-127.  Per key tile, h0/h1 score matmuls write the two halves of
    ONE [128, 1024] PSUM tile: the tile scheduler keeps same-tile
    matmuls adjacent, and disjoint row-groups make them concurrent
    (1 slot for 2 heads).  Head 2's Q^T/K^T are stored twice (both
    partition halves, built free by col-tiled projection matmuls), so
    the h2 scores for a key-tile PAIR also run concurrently.
  - S^T tiles keep KEYS on partitions: the key-padding mask and the
    1/sqrt(HD) scale are applied for free by the Exp activation
    (per-partition bias + scale).
  - V stored [T_k, 65] per head with a ones-column: the PV matmul
    accumulates [Y^T | softmax-denominator] in one pass.  PV for group
    i is emitted between the h01 and h2 scores of group i+1 so the PE
    streams while the ACT engine exps.
  - Mid-stream normalization is deferred into the NEXT chunk's g1 hook
    so its broadcast matmuls slot into a busy PE stream: stage Y'|denom
    to SBUF at the boundary, spread the denom row to [64,8] by DMA
    (DVE time scales with per-lane free size), reciprocal, DMA back,
    K=1 broadcast matmul, multiply.  The next chunk's Q projection is
    emitted at g1 too -- it doubles as PE filler where the PE would
    otherwise outrun the chunk's first exps.
  - TAIL normalization avoids the DMA hops entirely (two ~1.5us DGE+
    semaphore latencies would stall the drained PE): PE-transpose the
    denominator row into partitions ([1,512] -> [128,4] in pieces),
    reciprocal across 128 DVE lanes, PE-transpose back per piece into
    one [1,512] PSUM row, copy to SBUF, broadcast.  Stage-major
    emission across the three heads pipelines the chains; the tiny
    transposes keep the PE clock (HAM) from dropping to half rate.
  - Startup: the DMA order tracks the compute critical path (Wk, x
    chunk 0, Wq first; V/Wp later), small constants ride one packed
    [128, kk+4] tensor, the ones rows are memset on-device (memsets
    lead so the clock-ramping warmup matmuls start immediately).
  - Output-projection copies ride the DVE mid-stream (the ACT engine
    is exp-bound) and the ACT in the tail (the DVE does the normalize
    multiplies); the tail output DMA is striped over three queues.
Matmul operands are fp16 (full PE rate; fp32 PSUM accumulation).
"""

import itertools
import math
import os
import sys
from contextlib import ExitStack

import numpy as np

sys.path.insert(0, "/opt/trn_rl_repo")

import concourse.bass as bass  # noqa: E402,F401
import concourse.tile as tile  # noqa: E402
from concourse import bacc, mybir  # noqa: E402
from concourse import bass_utils  # noqa: E402

F32 = mybir.dt.float32
F16 = mybir.dt.float16
U16 = mybir.dt.uint16
ONE_F16 = 0x3C00  # 1.0 in fp16 bits (memset can't take fp16 directly)

B, T, C, H = 2, 2048, 768, 12
HD = C // H          # 64
GROUPS = 4           # head-groups (tensor parallel)
HPG = H // GROUPS    # 3 heads per group
J = HPG * HD         # 192 local channels
NCORES = 8
SCALE = 1.0 / math.sqrt(HD)
MASK_NEG = -30000.0  # exp(-30000 + small) == 0.0

COMPACT = os.environ.get("ATTN_NO_COMPACT", "") == ""
QCW = 512            # query chunk width for the attention phase


def _nchunks(n, cap=512, lo=256):
    """Split n (multiple of 128) into (start, width) chunks in [lo, cap]."""
    assert n % 128 == 0
    out, pos, rem = [], 0, n
    while rem > 0:
        w = min(cap, rem)
        if rem - w != 0 and rem - w < lo:
            w = max(lo, ((rem - lo) // 128) * 128)
        out.append((pos, w))
        pos += w
        rem -= w
    return out


def _pairing(kk):
    """Key tiles in pairs (last one single if kk is odd)."""
    gs = []
    i = 0
    while i + 1 < kk:
        gs.append((i, i + 1))
        i += 2
    if i < kk:
        gs.append((i,))
    return gs


def build_nc(tk, clean_kk=0):
    """Build the per-core Bass program.  tk = padded key count (mult of 128)."""
    kk = tk // 128

    nc = bacc.Bacc("TRN2", target_bir_lowering=False, debug=False)

    xt = nc.dram_tensor("xt", [768, T], F16, kind="ExternalInput").ap()
    wqT = nc.dram_tensor("wqT", [768, J], F16, kind="ExternalInput").ap()
    wkT = nc.dram_tensor("wkT", [768, J], F16, kind="ExternalInput").ap()
    wvT = nc.dram_tensor("wvT", [769, J], F16, kind="ExternalInput").ap()
    msk = nc.dram_tensor("msk", [128, kk + 4], F32, kind="ExternalInput").ap()
    eye = nc.dram_tensor("eye", [128, 128], F16, kind="ExternalInput").ap()
    wpT = nc.dram_tensor("wpT", [J + 1, 768], F16, kind="ExternalInput").ap()
    out = nc.dram_tensor("o", [T, 768], F16, kind="ExternalOutput").ap()

    with tile.TileContext(nc) as tc, ExitStack() as ctx:
        const = ctx.enter_context(tc.tile_pool(name="const", bufs=1))
        ppool = ctx.enter_context(tc.tile_pool(name="psum", bufs=4, space="PSUM"))
        stpool = ctx.enter_context(tc.tile_pool(name="stbig", bufs=2, space="PSUM"))
        espool = ctx.enter_context(tc.tile_pool(name="es", bufs=6))
        e2pool = ctx.enter_context(tc.tile_pool(name="es2", bufs=3))
        opool = ctx.enter_context(tc.tile_pool(name="osb", bufs=4))
        mpool = ctx.enter_context(tc.tile_pool(name="misc", bufs=3))

        # ---------------- persistent SBUF tensors ----------------
        xt_s = const.tile([128, 6, T], F16, tag="xt")
        xt1_s = const.tile([1, T], F16, tag="xt1")
        wq_s = const.tile([128, 6, J], F16, tag="wq")
        wk_s = const.tile([128, 6, J], F16, tag="wk")
        wv_s = const.tile([128, 6, J], F16, tag="wv")
        wv1_s = const.tile([1, J], F16, tag="wv1")
        msk_s = const.tile([128, kk + 4], F32, tag="msk")
        wpT01_s = const.tile([128, 768], F16, tag="wp01")
        wp2_s = const.tile([65, 768], F16, tag="wp2")
        ones_s = const.tile([65, 128], F16, tag="ones")
        onesf_s = const.tile([65, 8], F32, tag="onesf")
        eye_s = const.tile([128, 128], F16, tag="eye")
        # heads 0/1 packed on partitions 0-63 / 64-127; head 2 duplicated
        # on both partition halves (enables key-tile-pair concurrency)
        qt01_s = const.tile([128, T], F16, tag="qt01")
        qt2_s = const.tile([128, T], F16, tag="qt2")
        kt01_s = const.tile([128, tk], F16, tag="kt01")
        kt2_s = const.tile([128, tk], F16, tag="kt2")
        v_s = [const.tile([128, kk, 65], F16, tag=f"v{h}", name=f"v{h}") for h in range(3)]
        yn01_s = const.tile([128, T], F16, tag="yn01")
        yn2_s = const.tile([65, T], F16, tag="yn2")
        warm_s = const.tile([128, 512], F16, tag="warm")

        mb_s = msk_s[:, 0:kk]
        bq_s = msk_s[:, kk:kk + 2]
        bk_s = msk_s[:, kk + 2:kk + 4]

        # memsets first: they gate the warmup matmuls and depend on nothing
        nc.vector.memset(warm_s[:, :].bitcast(U16), 0)
        nc.vector.memset(ones_s[:, :].bitcast(U16), ONE_F16)
        nc.vector.memset(onesf_s[:, :], 1.0)
        nc.vector.memset(xt1_s[:, :].bitcast(U16), ONE_F16)
        nc.vector.memset(yn2_s[64:65, :].bitcast(U16), ONE_F16)
        for h in range(3):
            nc.vector.memset(v_s[h][:, :, 64:65].bitcast(U16), ONE_F16)
        # pre-load the ACT exp table while the input DMA streams (the
        # first real exp would otherwise pay the ~1.3us table load)
        wact = mpool.tile([64, 8], F16, tag="wact", name="wact")
        nc.scalar.activation(out=wact[:, :], in_=warm_s[0:64, 0:8],
                             func=mybir.ActivationFunctionType.Exp,
                             bias=0.0, scale=1.0)

        # ---------------- input DMAs ----------------
        # Ordered along the compute critical path: K proj needs wk + x[0:512],
        # Q proj chunk 0 needs wq; V/Wp only matter ~10us in.  The critical
        # first ~1.4MB is striped over FOUR issue queues (the DVE is idle
        # after the memsets); the rest stays off the DVE so it is free for
        # the projection copies.
        dq = itertools.cycle([nc.sync, nc.scalar, nc.gpsimd])

        def dma(dst, src):
            next(dq).dma_start(dst, src)

        dma4 = dma

        xt_r = xt.rearrange("(c p) t -> p c t", p=128)
        wq_r = wqT.rearrange("(c p) j -> p c j", p=128)
        wk_r = wkT.rearrange("(c p) j -> p c j", p=128)
        wv_r = wvT[0:768, :].rearrange("(c p) j -> p c j", p=128)

        def dma_x(n0, n1, f=None):
            """One x^T column range as two striped DMAs (c-blocks 0-2 / 3-5)."""
            f = f or dma
            f(xt_s[:, 0:3, n0:n1], xt_r[:, 0:3, n0:n1])
            f(xt_s[:, 3:6, n0:n1], xt_r[:, 3:6, n0:n1])

        up_w = min(tk, 512)
        dma4(wk_s[:, 0:3, :], wk_r[:, 0:3, :])
        dma4(wk_s[:, 3:6, :], wk_r[:, 3:6, :])
        nc.gpsimd.dma_start(msk_s[:, :], msk)
        # first 512 x columns in fine strips so the upfront projections
        # free-flow behind the DMA stream
        for (n0, nw) in _nchunks(up_w, cap=256, lo=128):
            dma_x(n0, n0 + nw, f=dma4)
        dma4(wq_s[:, 0:3, :], wq_r[:, 0:3, :])
        dma4(wq_s[:, 3:6, :], wq_r[:, 3:6, :])
        dma(wv_s[:, 0:3, :], wv_r[:, 0:3, :])
        dma(wv_s[:, 3:6, :], wv_r[:, 3:6, :])
        nc.gpsimd.dma_start(wv1_s[:, :], wvT[768:769, :])
        for (n0, nw) in _nchunks(tk - up_w) if tk > up_w else []:
            dma_x(up_w + n0, up_w + n0 + nw)
        if tk < T:
            for (n0, nw) in _nchunks(T - tk):
                dma_x(tk + n0, tk + n0 + nw)
        dma(wpT01_s[:, :], wpT[0:128, :])
        dma(wp2_s[:, :], wpT[128:193, :])
        nc.gpsimd.dma_start(eye_s[:, :], eye)

        # ---------------- projections (emitted just-in-time) ----------------
        def proj_qk(w_s, b_s, dst01, dst2, n0, nw):
            """Q^T/K^T for one t-chunk: out[j, t] = W.T[:, j].T @ x^T[:, t].
            The m=64 block (head 2) is computed twice via two col-tiled
            matmuls (array cols 0-63 / 64-127, concurrent) so dst2 holds
            the head-2 rows on BOTH partition halves.  PSUM->SBUF copies
            ride the DVE so the ACT engine stays free for the exp stream."""
            pt = ppool.tile([128, 512], F32, tag="ps", name="pt_qk")
            for ci in range(6):
                nc.tensor.matmul(
                    pt[0:128, 0:nw],
                    lhsT=w_s[:, ci, 0:128],
                    rhs=xt_s[:, ci, n0:n0 + nw],
                    start=(ci == 0), stop=(ci == 5))
            nc.vector.tensor_scalar(
                out=dst01[:, n0:n0 + nw], in0=pt[0:128, 0:nw],
                scalar1=b_s[:, 0:1], scalar2=None, op0=mybir.AluOpType.add)
            pt2 = ppool.tile([128, 512], F32, tag="ps", name="pt_qk2")
            for half in range(2):
                for ci in range(6):
                    nc.tensor.matmul(
                        pt2[64 * half:64 * half + 64, 0:nw],
                        lhsT=w_s[:, ci, 128:192],
                        rhs=xt_s[:, ci, n0:n0 + nw],
                        start=(ci == 0), stop=(ci == 5))
            nc.vector.tensor_scalar(
                out=dst2[:, n0:n0 + nw], in0=pt2[0:128, 0:nw],
                scalar1=b_s[:, 1:2], scalar2=None, op0=mybir.AluOpType.add)

        def proj_v(tt):
            pt = ppool.tile([128, 512], F32, tag="ps", name="pt_v")
            for ci in range(6):
                nc.tensor.matmul(
                    pt[:, 0:J],
                    lhsT=xt_s[:, ci, tt * 128:(tt + 1) * 128],
                    rhs=wv_s[:, ci, :],
                    start=(ci == 0), stop=False)
            nc.tensor.matmul(
                pt[:, 0:J],
                lhsT=xt1_s[0:1, tt * 128:(tt + 1) * 128],
                rhs=wv1_s[0:1, :],
                start=False, stop=True)
            for h in range(3):
                nc.vector.tensor_copy(v_s[h][:, tt, 0:64], pt[:, h * 64:(h + 1) * 64])

        spool = ctx.enter_context(tc.tile_pool(name="ystage", bufs=4))
        NQ = T // QCW

        def stage_chunk(yps):
            """Boundary: copy Y'|denom to SBUF, freeing the PSUM slots."""
            yss = []
            for h in range(3):
                ys = spool.tile([65, QCW], F32, tag="ys", name=f"ys{h}")
                nc.vector.tensor_copy(ys[:, :], yps[h][0:65, 0:QCW])
                yss.append(ys)
            return yss

        def spread_recip(src, nq):
            """Spread one head's [1,512] denominator row to [64,8] by DMA
            (reciprocal on 64 DVE lanes -- DVE time scales with per-lane
            free size), DMA back to row form for the broadcast matmul."""
            dsp = mpool.tile([64, 8], F32, tag="dsp", name="dsp")
            next(nq).dma_start(dsp[:, :], src)
            rsp = mpool.tile([64, 8], F16, tag="rsp", name="rsp")
            with nc.allow_low_precision(reason="1/denom consumed as fp16"):
                nc.vector.reciprocal(rsp[:, :], dsp[:, :])
            rd = mpool.tile([65, QCW], F16, tag="rd", name="rd")
            next(nq).dma_start(rd[64:65, :], rsp[:, :])
            return rd

        def bcasts(rd3, bc01, bc2):
            for h in range(2):
                nc.tensor.matmul(
                    bc01[64 * h:64 * h + 64, 0:QCW],
                    lhsT=ones_s[64:65, 0:64],
                    rhs=rd3[h][64:65, 0:QCW],
                    start=True, stop=True)
            nc.tensor.matmul(
                bc2[0:64, 0:QCW],
                lhsT=ones_s[64:65, 0:64],
                rhs=rd3[2][64:65, 0:QCW],
                start=True, stop=True)

        def emit_normalize(qc, yss):
            """Deferred into the next chunk (g1 hook) so the broadcast
            matmuls slot into a busy PE stream instead of stalling it."""
            q0 = qc * QCW
            nq = itertools.cycle([nc.gpsimd, nc.sync, nc.scalar])
            rd3 = [spread_recip(yss[h][64:65, :], nq) for h in range(3)]
            bc01 = ppool.tile([128, 512], F32, tag="ps", name="bc01")
            bc2 = ppool.tile([128, 512], F32, tag="ps", name="bc2")
            bcasts(rd3, bc01, bc2)
            for h in range(3):
                yn_ap = (yn01_s[64 * h:64 * h + 64, q0:q0 + QCW] if h < 2
                         else yn2_s[0:64, q0:q0 + QCW])
                bc_ap = (bc01[64 * h:64 * h + 64, 0:QCW] if h < 2
                         else bc2[0:64, 0:QCW])
                nc.vector.tensor_tensor(
                    out=yn_ap, in0=yss[h][0:64, :], in1=bc_ap,
                    op=mybir.AluOpType.mult)

        def oproj_tile(tt, tail=False, oq=None):
            """Output projection for one 128-row t-tile.  PSUM->SBUF copies
            alternate DVE/ACT; the output DMA rides one queue mid-stream,
            three striped queues in the tail."""
            o_sb = opool.tile([128, 768], F16, tag="osb", name="o_sb")
            for oi, (n0, nw) in enumerate(
                    _nchunks(768, cap=256) if tail else _nchunks(768)):
                op = ppool.tile([128, 512], F32, tag="ps", name="op")
                nc.tensor.matmul(
                    op[0:128, 0:nw],
                    lhsT=yn01_s[:, tt * 128:(tt + 1) * 128],
                    rhs=wpT01_s[:, n0:n0 + nw],
                    start=True, stop=False)
                nc.tensor.matmul(
                    op[0:128, 0:nw],
                    lhsT=yn2_s[:, tt * 128:(tt + 1) * 128],
                    rhs=wp2_s[:, n0:n0 + nw],
                    start=False, stop=True)
                # mid-stream the ACT engine is the busy one (exp stream), so
                # copies ride the DVE; in the tail the DVE does the
                # normalize multiplies, so copies ride the ACT
                if tail:
                    nc.scalar.copy(o_sb[:, n0:n0 + nw], op[0:128, 0:nw])
                else:
                    nc.vector.tensor_copy(o_sb[:, n0:n0 + nw], op[0:128, 0:nw])
                eng = next(oq) if tail else nc.sync
                eng.dma_start(
                    out[tt * 128:(tt + 1) * 128, n0:n0 + nw],
                    o_sb[:, n0:n0 + nw])

        def final_groups(qc):
            return [(lambda tt=tt: oproj_tile(tt))
                    for tt in range(qc * (QCW // 128), (qc + 1) * (QCW // 128))]

        def warmup(n, read_back=False):
            """n dummy full-array matmuls on a zero tile: fills dependency
            gaps so the HAM clock gate stays at full rate.  read_back adds
            a tiny DVE read so the BIR verifier sees a consumer."""
            wp_ps = ppool.tile([128, 512], F32, tag="ps", name="warm_ps")
            for i in range(n):
                nc.tensor.matmul(wp_ps[:, 0:512], lhsT=warm_s[:, 0:128],
                                 rhs=warm_s[:, 0:512], start=True, stop=True)
            if read_back:
                junk = mpool.tile([1, 8], F16, tag="junk", name="junk")
                with nc.allow_low_precision(reason="dummy read"):
                    nc.vector.tensor_copy(junk[:, :], wp_ps[0:1, 0:8])

        # upfront: only what the first q-chunk needs immediately, in DMA
        # arrival order (K chunk halves, Q chunk halves, then V tiles)
        warmup(13)
        for (n0, nw) in _nchunks(up_w, cap=128, lo=128):
            proj_qk(wk_s, bk_s, kt01_s, kt2_s, n0, nw)
        kpend = [(up_w + n0, nw) for (n0, nw) in
                 (_nchunks(tk - up_w) if tk > up_w else [])]
        for (n0, nw) in _nchunks(min(T, 512), cap=256, lo=128):
            proj_qk(wq_s, bq_s, qt01_s, qt2_s, n0, nw)
        q_done = 1
        for tt in range(min(4, kk)):
            proj_v(tt)
        v_done = min(4, kk)

        qchunks = [(i * QCW, QCW) for i in range(NQ)]
        groups = _pairing(kk)
        filler = []
        pend_pv = None
        pend_norm = None

        def emit_pv():
            nonlocal pend_pv
            if pend_pv is None:
                return
            g, esd, yp = pend_pv
            pend_pv = None
            for h in range(3):
                for i, kkt in enumerate(g):
                    nc.tensor.matmul(
                        yp[h][0:65, 0:QCW],
                        lhsT=v_s[h][:, kkt, :],
                        rhs=esd[(h, i)],
                        start=(kkt == 0), stop=(kkt == kk - 1))

        for qc in range(NQ):
            q0 = qc * QCW
            pop_slots = {0, 2, 3, 4}
            yps = {}
            for h in range(3):
                yps[h] = ppool.tile([128, 512], F32, tag="ps", name=f"yp{h}")
            for gi, g in enumerate(groups):
                kt0 = g[0]
                # just-in-time remaining projections (first q-chunk only)
                while kpend and kpend[0][0] < (kt0 + 4) * 128:
                    proj_qk(wk_s, bk_s, kt01_s, kt2_s, *kpend.pop(0))
                while v_done < kk and v_done <= kt0 + 3:
                    proj_v(v_done)
                    v_done += 1
                # the next chunk's Q projection doubles as PE filler: at g1
                # the PE otherwise outruns the chunk's first exps (for
                # chunk 0 the x columns may not have landed yet, so g3)
                if (gi == (3 if qc == 0 else 1)
                        and q_done <= qc + 1 and qc + 1 < NQ):
                    proj_qk(wq_s, bq_s, qt01_s, qt2_s, *qchunks[qc + 1])
                    q_done = qc + 2
                # scores: per key tile, heads 0/1 write the two halves of ONE
                # PSUM tile -> the scheduler keeps them adjacent and the
                # disjoint row-groups run them concurrently (1 slot / tile)
                sts = {}
                for i, kkt in enumerate(g):
                    st = stpool.tile([128, 1024], F32, tag="stb", name=f"st_{i}")
                    sts[i] = st
                    for h in range(2):
                        rows = slice(64 * h, 64 * h + 64)
                        nc.tensor.matmul(
                            st[:, 512 * h:512 * h + 512],
                            lhsT=kt01_s[rows, kkt * 128:(kkt + 1) * 128],
                            rhs=qt01_s[rows, q0:q0 + QCW],
                            start=True, stop=True)
                # previous group's PV rides here so the PE keeps streaming
                # while the ACT engine exps this group's scores
                emit_pv()
                # head-2 scores: the two key tiles of a pair use disjoint
                # partition halves of the duplicated K2/Q2 -> concurrent
                st2 = stpool.tile([128, 1024], F32, tag="stb", name="st2")
                for i, kkt in enumerate(g):
                    rows = slice(64 * i, 64 * i + 64)
                    nc.tensor.matmul(
                        st2[:, i * 512:(i + 1) * 512],
                        lhsT=kt2_s[rows, kkt * 128:(kkt + 1) * 128],
                        rhs=qt2_s[rows, q0:q0 + QCW],
                        start=True, stop=True)
                if gi == 1 and pend_norm is not None:
                    nqc, yss_p = pend_norm
                    pend_norm = None
                    emit_normalize(nqc, yss_p)
                    filler.extend(final_groups(nqc))
                elif gi in pop_slots and filler:
                    filler.pop(0)()
                esd = {}
                last_g = (qc == NQ - 1 and gi == len(groups) - 1)
                for i, kkt in enumerate(g):
                    # one wide exp covers both heads' halves: the mask bias
                    # depends only on the key tile, shared by h0/h1.  For
                    # the very last group the halves are separate ops so
                    # the tail's first PV starts half an exp earlier
                    es = espool.tile([128, 1024], F16, tag="es")
                    if last_g:
                        for hh in range(2):
                            nc.scalar.activation(
                                out=es[:, 512 * hh:512 * hh + 512],
                                in_=sts[i][:, 512 * hh:512 * hh + 512],
                                func=mybir.ActivationFunctionType.Exp,
                                bias=mb_s[:, kkt:kkt + 1], scale=SCALE)
                    else:
                        nc.scalar.activation(
                            out=es[:, :], in_=sts[i][:, :],
                            func=mybir.ActivationFunctionType.Exp,
                            bias=mb_s[:, kkt:kkt + 1], scale=SCALE)
                    for h in range(2):
                        esd[(h, i)] = es[:, 512 * h:512 * h + 512]
                es2 = e2pool.tile([128, 1024], F16, tag="es2")
                if g[-1] < clean_kk:
                    nc.scalar.activation(
                        out=es2[:, 0:512 * len(g)], in_=st2[:, 0:512 * len(g)],
                        func=mybir.ActivationFunctionType.Exp,
                        bias=0.0, scale=SCALE)
                else:
                    for i, kkt in enumerate(g):
                        nc.scalar.activation(
                            out=es2[:, i * 512:(i + 1) * 512],
                            in_=st2[:, i * 512:(i + 1) * 512],
                            func=mybir.ActivationFunctionType.Exp,
                            bias=mb_s[:, kkt:kkt + 1], scale=SCALE)
                for i in range(len(g)):
                    esd[(2, i)] = es2[:, i * 512:(i + 1) * 512]
                pend_pv = (g, esd, yps)
            if qc == NQ - 1:
                break
            emit_pv()
            if q_done <= qc + 1 and qc + 1 < NQ:
                proj_qk(wq_s, bq_s, qt01_s, qt2_s, *qchunks[qc + 1])
                q_done = qc + 2
            # stage Y' out of PSUM at the boundary; the normalize chain and
            # oproj closures are deferred into the next chunk's g1 hook
            pend_norm = (qc, stage_chunk(yps))
        # ---------------- tail ----------------
        # the last group's PV is emitted PER HEAD with that head's
        # reciprocal right behind it; one stpool tile (free once the last
        # exp retired) holds all three broadcasts; the yn multiplies are
        # split per 128-query tile so each output-projection tile starts
        # as soon as its slice is ready.  Dummy matmuls bridge the chain
        # latency so the PE clock stays at full rate.
        g, esd, yp = pend_pv
        pend_pv = None
        qc = NQ - 1
        q0 = qc * QCW

        # Tail normalize with NO DMA hops, stage-major across heads so the
        # per-head chains pipeline instead of serializing in the in-order
        # PE queue: PE-transpose each denominator row into partitions
        # ([1,512] -> [128,4] in pieces), one reciprocal across 128 DVE
        # lanes per head, PE-transpose back to a [1,512] PSUM row, copy to
        # SBUF, broadcast.  Engine-to-engine latency only, and the PE work
        # keeps the HAM clock hot through the tail.
        for h in range(3):
            for i, kkt in enumerate(g):
                nc.tensor.matmul(
                    yp[h][0:65, 0:QCW],
                    lhsT=v_s[h][:, kkt, :],
                    rhs=esd[(h, i)],
                    start=(kkt == 0), stop=(kkt == kk - 1))
        yss = []
        for h in range(3):
            ys = spool.tile([65, QCW], F32, tag="ys", name=f"tys{h}")
            # h2 is the chain-critical head (its PV ends last): its stage
            # copy rides the ACT engine, idle once the last exp retires
            if h == 2:
                nc.scalar.copy(ys[:, :], yp[h][0:65, 0:QCW])
            else:
                nc.vector.tensor_copy(ys[:, :], yp[h][0:65, 0:QCW])
            yss.append(ys)
        while filler:
            filler.pop(0)()
        # h2 first throughout: its PV ends last, so its chain gates the
        # first output-projection tile; its rdt copy rides the idle ACT
        # engine so it never queues behind h0/h1's DVE work
        tps, rsps, tbrows, rdts = {}, {}, {}, {}
        for h in (2, 0, 1):
            tp = ppool.tile([128, 512], F32, tag="ps", name=f"tp{h}")
            for p in range(QCW // 128):
                nc.tensor.transpose(tp[0:128, p:p + 1],
                                    yss[h][64:65, p * 128:(p + 1) * 128],
                                    onesf_s[64:65, 0:1])
            tps[h] = tp
        for h in (2, 0, 1):
            rsp = mpool.tile([128, 4], F16, tag="rsp", name=f"rsp{h}")
            with nc.allow_low_precision(reason="1/denom consumed as fp16"):
                nc.vector.reciprocal(rsp[:, :], tps[h][0:128, 0:QCW // 128])
            rsps[h] = rsp
        for h in (2, 0, 1):
            tbrow = ppool.tile([1, 512], F16, tag="ps", name=f"tbrow{h}")
            for p in range(QCW // 128):
                nc.tensor.transpose(tbrow[0:1, p * 128:(p + 1) * 128],
                                    rsps[h][0:128, p:p + 1],
                                    eye_s[0:128, 0:128])
            tbrows[h] = tbrow
        for h in (2, 0, 1):
            rdt = mpool.tile([1, QCW], F16, tag="rdt", name=f"rdt{h}")
            if h == 2:
                nc.scalar.copy(rdt[0:1, :], tbrows[h][0:1, 0:QCW])
            else:
                nc.vector.tensor_copy(rdt[0:1, :], tbrows[h][0:1, 0:QCW])
            rdts[h] = rdt
        bc01 = ppool.tile([128, 512], F32, tag="ps", name="tbc01")
        bc2 = ppool.tile([128, 512], F32, tag="ps", name="tbc2")
        # all three broadcasts back-to-back (h2 first: its rdt lands
        # last), then the multiplies in 256-wide pieces
        def bc_of(h):
            return bc01[64 * h:64 * h + 64, :] if h < 2 else bc2[0:64, :]

        for h in (2, 0, 1):
            nc.tensor.matmul(
                bc_of(h)[:, 0:QCW],
                lhsT=ones_s[0:1, 0:64],
                rhs=rdts[h][0:1, 0:QCW],
                start=True, stop=True)
        # redundant broadcasts into a junk tile bridge the ~1us window
        # where the PE waits for the first multiply pieces; anchored on
        # rdt so the scheduler cannot hoist them earlier
        jb = ppool.tile([128, 512], F32, tag="ps", name="tjb")
        for i in range(6):
            nc.tensor.matmul(
                jb[0:64, 0:QCW],
                lhsT=ones_s[0:1, 0:64],
                rhs=rdts[i % 3][0:1, 0:QCW],
                start=True, stop=True)
        junk = mpool.tile([1, 8], F16, tag="junk", name="tjunk")
        with nc.allow_low_precision(reason="dummy read"):
            nc.vector.tensor_copy(junk[:, :], jb[0:1, 0:8])
        # multiplies in 128-wide pieces, head-minor, with each output-
        # projection tile right behind its three pieces: the first tile's
        # gate is 3 small DVE ops instead of the full multiply sweep
        toq = itertools.cycle([nc.gpsimd, nc.scalar, nc.sync])
        for tp in range(QCW // 128):
            c0 = tp * 128
            for h in (2, 0, 1):
                yn_s = yn01_s if h < 2 else yn2_s
                r0 = 64 * h if h < 2 else 0
                nc.vector.tensor_tensor(
                    out=yn_s[r0:r0 + 64, q0 + c0:q0 + c0 + 128],
                    in0=yss[h][0:64, c0:c0 + 128],
                    in1=bc_of(h)[:, c0:c0 + 128], op=mybir.AluOpType.mult)
            oproj_tile(qc * (QCW // 128) + tp, tail=True, oq=toq)

    nc.compile()
    return nc


def _prep_core_inputs(x, attn_mask1, Wq, bq, Wk, bk, Wv, bv, Wp, bp):
    """Host-side sharding: returns (in_maps, tk, clean_kk, perms)."""
    x = np.asarray(x, np.float32)
    attn_mask1 = np.asarray(attn_mask1)
    Wq, Wk, Wv, Wp = (np.asarray(a, np.float32) for a in (Wq, Wk, Wv, Wp))
    bq, bk, bv, bp = (np.asarray(a, np.float32) for a in (bq, bk, bv, bp))

    if COMPACT:
        # permute tokens so unmasked keys come first; Q/K/V share one x^T
        idxs = [np.nonzero(attn_mask1[b] != 0)[0] for b in range(B)]
        perms = [np.concatenate([idxs[b], np.nonzero(attn_mask1[b] == 0)[0]])
                 for b in range(B)]
        nmax = max(max(len(i) for i in idxs), 1)
        tk = min(((nmax + 127) // 128) * 128, T)
        clean_kk = min(len(i) for i in idxs) // 128
        mbs = []
        for b in range(B):
            m = np.zeros(tk, np.float32)
            m[len(idxs[b]):] = MASK_NEG
            mbs.append(m)
    else:
        perms = [np.arange(T), np.arange(T)]
        tk = T
        clean_kk = 0
        mbs = [np.where(attn_mask1[b] != 0, 0.0, MASK_NEG).astype(np.float32)
               for b in range(B)]
    kk = tk // 128
    xts = [x[b][perms[b]].T.astype(np.float16) for b in range(B)]

    WqT, WkT, WvT, WpT = (W.T.astype(np.float16) for W in (Wq, Wk, Wv, Wp))

    in_maps = []
    for c in range(NCORES):
        b, g = c // GROUPS, c % GROUPS
        js = slice(g * J, (g + 1) * J)
        # packed small constants: mask bias columns, then bq/bk in the
        # [128, 2] layout the projections read (second col duplicated on
        # both partition halves)
        mskb = np.zeros((128, kk + 4), np.float32)
        mskb[:, 0:kk] = mbs[b].reshape(kk, 128).T
        for ci, bias in ((kk, bq[js]), (kk + 2, bk[js])):
            mskb[:, ci] = bias[0:128]
            mskb[0:64, ci + 1] = bias[128:192]
            mskb[64:128, ci + 1] = bias[128:192]
        m = {
            "xt": xts[b],
            "eye": np.eye(128, dtype=np.float16),
            "wqT": np.ascontiguousarray(WqT[:, js]),
            "wkT": np.ascontiguousarray(WkT[:, js]),
            "wvT": np.concatenate([WvT[:, js], bv[js].astype(np.float16)[None, :]], axis=0),
            "msk": mskb,
            "wpT": np.concatenate([WpT[js, :],
                                   (bp / GROUPS).astype(np.float16)[None, :]], axis=0),
        }
        in_maps.append(m)
    return in_maps, tk, clean_kk, perms


_CACHE = {}


def kernel(**inputs):
    in_maps, tk, clean_kk, perms = _prep_core_inputs(**inputs)
    key = (tk, clean_kk)
    if key not in _CACHE:
        _CACHE[key] = build_nc(tk, clean_kk)
    nc = _CACHE[key]
    res = bass_utils.run_bass_kernel_spmd(nc, in_maps, list(range(NCORES)))
    out = np.zeros((B, T, C), np.float32)
    for c in range(NCORES):
        out[c // GROUPS][perms[c // GROUPS]] += res.results[c]["o"].astype(np.float32)
    return out


if __name__ == "__main__":
    rng = np.random.default_rng(0)
    ins = {
        "x": rng.standard_normal((B, T, C), dtype=np.float32),
        "attn_mask1": rng.integers(0, 2, size=(B, T)).astype(np.int32),
        "Wq": rng.standard_normal((C, C), dtype=np.float32) * 0.02,
        "bq": np.zeros(C, np.float32),
        "Wk": rng.standard_normal((C, C), dtype=np.float32) * 0.02,
        "bk": np.zeros(C, np.float32),
        "Wv": rng.standard_normal((C, C), dtype=np.float32) * 0.02,
        "bv": np.zeros(C, np.float32),
        "Wp": rng.standard_normal((C, C), dtype=np.float32) * 0.02,
        "bp": np.zeros(C, np.float32),
    }
    out = kernel(**ins)
    print(out.shape, out.dtype, np.abs(out).max())
